# revision 9
# baseline (speedup 1.0000x reference)
"""Trainium2 Bass kernel for nn_ASISNativeAttention (B=2,S=2048,D=1024,H=16).

Sharding: 8 cores = 2 batches x 4 head-groups (4 heads each).
Each core computes, for its batch b and heads [4g, 4g+4):
  qT,kT  [256, 2048]  (transposed projections, bias + 1/8 scale folded into q)
  v      [2048, 4x65] (natural projection + bias, with a ones column per head)
  scoresT[sk, sq] = kT_h^T-slices; exp via ACT (no max pass -- scores bounded)
  ctx_aug[sq, 65] = sum_sk expT * v_aug  (col 64 = softmax denominator)
  ctx    = ctx_aug[:, :64] * recip(denominator)        (per-partition scalar)
  ctxT   = transpose(ctx) * gate_col (ethics*safety broadcast over 64 rows)
  out_partial = ctxT^T-slices @ Wo_slice               [2048, 1024]
Host: out[b] = sum_g partial + bo.

All device tensors are pre-laid-out on the host so every DMA is contiguous
per partition.
"""

import os
import sys
import numpy as np

sys.path.insert(0, "/opt/trn_rl_repo")

B, S, D, H = 2, 2048, 1024, 16
HD = 64          # head dim
NCORES = 8
HG = 4           # head groups = cores per batch
HL = H // HG     # heads per core (4)
DL = D // HG     # local width (256)
ST = S // 128    # 16 s-tiles
DT = D // 128    # 8 d-tiles
SC = 512         # sq chunk width for scores
NSC = S // SC    # 4 chunks

_CACHE = {}


def _build_nc():
    import concourse.bass as bass
    import concourse.mybir as mybir
    from concourse.tile import TileContext

    fp32 = mybir.dt.float32
    AF = mybir.ActivationFunctionType
    ALU = mybir.AluOpType

    nc = bass.Bass()

    x_d = nc.declare_dram_parameter("x", [S, D], fp32, isOutput=False)
    wq_d = nc.declare_dram_parameter("wq", [128, DT * DL], fp32, isOutput=False)
    wk_d = nc.declare_dram_parameter("wk", [128, DT * DL], fp32, isOutput=False)
    wv_d = nc.declare_dram_parameter("wv", [128, DT * DL], fp32, isOutput=False)
    wo_d = nc.declare_dram_parameter("wo", [128, 2 * D], fp32, isOutput=False)
    bq_d = nc.declare_dram_parameter("bq2", [128, 2], fp32, isOutput=False)
    bk_d = nc.declare_dram_parameter("bk2", [128, 2], fp32, isOutput=False)
    bv_d = nc.declare_dram_parameter("bvrow", [1, DL], fp32, isOutput=False)
    wes_d = nc.declare_dram_parameter("wes", [128, DT * 2 * HL], fp32, isOutput=False)
    bes_d = nc.declare_dram_parameter("bes", [HL, 2], fp32, isOutput=False)
    gexp_d = nc.declare_dram_parameter("gexp", [HL, DL], fp32, isOutput=False)
    id_d = nc.declare_dram_parameter("ident", [128, 128], fp32, isOutput=False)
    out_d = nc.declare_dram_parameter("out", [S, D], fp32, isOutput=True)

    with TileContext(nc) as tc:
        with tc.tile_pool(name="persist", bufs=1) as P:
            # ---- persistent SBUF tensors ----
            xT = P.tile([128, DT * S], fp32, tag="xT")        # 64KB/part
            wq = P.tile([128, DT * DL], fp32, tag="wq")
            wk = P.tile([128, DT * DL], fp32, tag="wk")
            wv = P.tile([128, DT * DL], fp32, tag="wv")
            wo = P.tile([128, 2 * D], fp32, tag="wo")
            qT = P.tile([128, 2 * S], fp32, tag="qT")
            kT = P.tile([128, 2 * S], fp32, tag="kT")
            v = P.tile([128, ST * HL * 65], fp32, tag="v")
            ctx = P.tile([128, ST * DL], fp32, tag="ctx")
            ctxT = P.tile([128, 2 * S], fp32, tag="ctxT")
            bq2 = P.tile([128, 2], fp32, tag="bq2")
            bk2 = P.tile([128, 2], fp32, tag="bk2")
            bvrow = P.tile([1, DL], fp32, tag="bvrow")
            wes = P.tile([128, DT * 2 * HL], fp32, tag="wes")
            bes = P.tile([HL, 2], fp32, tag="bes")
            gexp = P.tile([HL, DL], fp32, tag="gexp")
            ident = P.tile([128, 128], fp32, tag="ident")
            ones_row = P.tile([1, 128], fp32, tag="ones_row")
            xm_parts = P.tile([128, DT * ST], fp32, tag="xm_parts")
            xm_col = P.tile([128, DT], fp32, tag="xm_col")
            gcol = P.tile([128, 2], fp32, tag="gcol")
            dummy = P.tile([128, ST], fp32, tag="dummy")

            dma = nc.sync.dma_start
            dma(wq[:], wq_d[:])
            dma(wk[:], wk_d[:])
            dma(wv[:], wv_d[:])
            dma(wo[:], wo_d[:])
            dma(bq2[:], bq_d[:])
            dma(bk2[:], bk_d[:])
            dma(bvrow[:], bv_d[:])
            dma(wes[:], wes_d[:])
            dma(bes[:], bes_d[:])
            dma(gexp[:], gexp_d[:])
            dma(ident[:], id_d[:])
            nc.vector.memset(ones_row[:], 1.0)

            # ---- phase 1: load x, transpose to xT, accumulate mean partials ----
            with (
                tc.tile_pool(name="xload", bufs=3) as XL,
                tc.tile_pool(name="ptp", bufs=4, space="PSUM") as PTP,
            ):
                for t in range(ST):
                    xt = XL.tile([128, D], fp32, tag="xt")
                    dma(xt[:], x_d[t * 128:(t + 1) * 128, :])
                    for j in range(DT):
                        ps = PTP.tile([128, 128], fp32, tag="tp")
                        nc.tensor.transpose(ps[:], xt[:, j * 128:(j + 1) * 128], ident[:])
                        nc.vector.tensor_scalar(
                            out=xT[:, j * S + t * 128: j * S + (t + 1) * 128],
                            in0=ps[:],
                            scalar1=1.0,
                            scalar2=None,
                            op0=ALU.mult,
                            op1=ALU.add,
                            accum_out=xm_parts[:, j * ST + t: j * ST + t + 1],
                        )

            # ---- phase 2: QKV projections + gate scalars ----
            with (
                tc.tile_pool(name="pqk", bufs=3, space="PSUM") as PQ,
                tc.tile_pool(name="pv", bufs=2, space="PSUM") as PVp,
                tc.tile_pool(name="pg", bufs=1, space="PSUM") as PG,
                tc.tile_pool(name="gs", bufs=1) as GS,
            ):
                # mean columns: xm_col[:, j] = sum(xm_parts[:, j*ST:(j+1)*ST]) / S
                for j in range(DT):
                    nc.scalar.activation(
                        out=dummy[:, 0:ST],
                        in_=xm_parts[:, j * ST:(j + 1) * ST],
                        func=AF.Copy,
                        scale=1.0 / S,
                        accum_out=xm_col[:, j: j + 1],
                    )

                # qT / kT: lhsT = W tile [128,128], rhs = xT chunk [128,512]
                for name, w, dst, bias, scl in (
                    ("q", wq, qT, bq2, 0.125),
                    ("k", wk, kT, bk2, 1.0),
                ):
                    for i in range(2):
                        for sc in range(NSC):
                            pp = PQ.tile([128, SC], fp32, tag="pqk")
                            for j in range(DT):
                                nc.tensor.matmul(
                                    pp[:],
                                    lhsT=w[:, j * DL + i * 128: j * DL + (i + 1) * 128],
                                    rhs=xT[:, j * S + sc * SC: j * S + (sc + 1) * SC],
                                    start=(j == 0),
                                    stop=(j == DT - 1),
                                )
                            nc.vector.tensor_scalar(
                                out=dst[:, i * S + sc * SC: i * S + (sc + 1) * SC],
                                in0=pp[:],
                                scalar1=bias[:, i: i + 1],
                                scalar2=scl,
                                op0=ALU.add,
                                op1=ALU.mult,
                            )

                # v natural: per s-tile [128, 256] + bias via rank-1 matmul
                for t in range(ST):
                    pv = PVp.tile([128, DL], fp32, tag="pv")
                    nc.tensor.matmul(
                        pv[:], lhsT=ones_row[:], rhs=bvrow[:],
                        start=True, stop=False,
                    )
                    for j in range(DT):
                        nc.tensor.matmul(
                            pv[:],
                            lhsT=xT[:, j * S + t * 128: j * S + (t + 1) * 128],
                            rhs=wv[:, j * DL:(j + 1) * DL],
                            start=False,
                            stop=(j == DT - 1),
                        )
                    vt = v[:, t * HL * 65:(t + 1) * HL * 65]
                    nc.vector.memset(
                        vt.rearrange("p (h c) -> p h c", c=65)[:, :, 64:65], 1.0
                    )
                    nc.vector.tensor_copy(
                        vt.rearrange("p (h c) -> p h c", c=65)[:, :, 0:64],
                        pv.rearrange("p (h c) -> p h c", c=64)[:, :, :],
                    )

                # gates: ethics/safety [4,1] -> gate -> broadcast columns
                gpe = PG.tile([HL, 1], fp32, tag="gpe")
                gps = PG.tile([HL, 1], fp32, tag="gps")
                for j in range(DT):
                    nc.tensor.matmul(
                        gpe[:], lhsT=wes[:, j * 8: j * 8 + 4],
                        rhs=xm_col[:, j: j + 1],
                        start=(j == 0), stop=(j == DT - 1),
                    )
                for j in range(DT):
                    nc.tensor.matmul(
                        gps[:], lhsT=wes[:, j * 8 + 4: j * 8 + 8],
                        rhs=xm_col[:, j: j + 1],
                        start=(j == 0), stop=(j == DT - 1),
                    )
                eth = GS.tile([HL, 1], fp32, tag="eth")
                saf = GS.tile([HL, 1], fp32, tag="saf")
                gate = GS.tile([HL, 1], fp32, tag="gate")
                nc.scalar.activation(eth[:], gpe[:], AF.Sigmoid, bias=bes[:, 0:1])
                nc.scalar.activation(saf[:], gps[:], AF.Sigmoid, bias=bes[:, 1:2])
                nc.vector.tensor_mul(gate[:], eth[:], saf[:])
                for i in range(2):
                    pgc = PG.tile([128, 1], fp32, tag="pgc")
                    nc.tensor.matmul(
                        pgc[:], lhsT=gexp[:, i * 128:(i + 1) * 128], rhs=gate[:],
                        start=True, stop=True,
                    )
                    nc.vector.tensor_copy(gcol[:, i: i + 1], pgc[:])

            # ---- phase 3: attention ----
            with (
                tc.tile_pool(name="ps", bufs=2, space="PSUM") as PS,
                tc.tile_pool(name="pc", bufs=5, space="PSUM") as PC,
                tc.tile_pool(name="ex", bufs=3) as EX,
                tc.tile_pool(name="rc", bufs=8) as RC,
            ):
                for h in range(HL):
                    i, r = h // 2, (h % 2) * 64
                    for sc in range(NSC):
                        cps = [PC.tile([128, 65], fp32, tag="cp", name=f"cp{h}_{sc}_{u}")
                               for u in range(SC // 128)]
                        for sk in range(ST):
                            sp = PS.tile([128, SC], fp32, tag="sp")
                            nc.tensor.matmul(
                                sp[:],
                                lhsT=kT[r:r + 64, i * S + sk * 128: i * S + (sk + 1) * 128],
                                rhs=qT[r:r + 64, i * S + sc * SC: i * S + (sc + 1) * SC],
                                start=True, stop=True,
                            )
                            et = EX.tile([128, SC], fp32, tag="et")
                            nc.scalar.activation(et[:], sp[:], AF.Exp)
                            for u in range(SC // 128):
                                nc.tensor.matmul(
                                    cps[u][:],
                                    lhsT=et[:, u * 128:(u + 1) * 128],
                                    rhs=v[:, sk * HL * 65 + h * 65: sk * HL * 65 + (h + 1) * 65],
                                    start=(sk == 0),
                                    stop=(sk == ST - 1),
                                )
                        for u in range(SC // 128):
                            t = sc * (SC // 128) + u
                            rec = RC.tile([128, 1], fp32, tag="rec")
                            nc.vector.reciprocal(rec[:], cps[u][:, 64:65])
                            nc.vector.tensor_scalar(
                                out=ctx[:, t * DL + h * HD: t * DL + (h + 1) * HD],
                                in0=cps[u][:, 0:HD],
                                scalar1=rec[:],
                                scalar2=None,
                                op0=ALU.mult,
                            )

            # ---- phase 4: transpose ctx (gate folded in), output projection ----
            with (
                tc.tile_pool(name="pt2", bufs=4, space="PSUM") as PT2,
                tc.tile_pool(name="po", bufs=2, space="PSUM") as PO,
                tc.tile_pool(name="ob", bufs=3) as OB,
            ):
                for i in range(2):
                    for t in range(ST):
                        ps = PT2.tile([128, 128], fp32, tag="tp2")
                        nc.tensor.transpose(
                            ps[:], ctx[:, t * DL + i * 128: t * DL + (i + 1) * 128],
                            ident[:],
                        )
                        nc.vector.tensor_scalar(
                            out=ctxT[:, i * S + t * 128: i * S + (t + 1) * 128],
                            in0=ps[:],
                            scalar1=gcol[:, i: i + 1],
                            scalar2=None,
                            op0=ALU.mult,
                        )
                for t in range(ST):
                    ot = OB.tile([128, D], fp32, tag="ot")
                    for n in range(2):
                        po = PO.tile([128, 512], fp32, tag="po")
                        for i in range(2):
                            nc.tensor.matmul(
                                po[:],
                                lhsT=ctxT[:, i * S + t * 128: i * S + (t + 1) * 128],
                                rhs=wo[:, i * D + n * 512: i * D + (n + 1) * 512],
                                start=(i == 0), stop=(i == 1),
                            )
                        nc.vector.tensor_copy(ot[:, n * 512:(n + 1) * 512], po[:])
                    dma(out_d[t * 128:(t + 1) * 128, :], ot[:])

    _split_multi_waits(nc)
    return nc


def _split_multi_waits(nc, skip=("InstEventSemaphore",)):
    """Hoist extra sync waits onto preceding same-engine NoOps.

    Walrus codegen can attach only one sync wait to some instruction
    encodings (e.g. the PE LDWEIGHTS struct), so any instruction carrying
    N>1 waits is rewritten as N-1 single-wait NoOps followed by the
    instruction with the last wait.
    """
    import concourse.mybir as mybir

    eng = {
        "EngineType.PE": nc.tensor,
        "EngineType.DVE": nc.vector,
        "EngineType.Activation": nc.scalar,
        "EngineType.Pool": nc.gpsimd,
        "EngineType.SP": nc.sync,
    }

    def fresh_nop(engine_key):
        nop = eng[engine_key].nop(hint="wsplit").ins
        for fn in nc.m.functions:
            for bb in fn.blocks:
                for i, ins in enumerate(bb.instructions):
                    if ins.name == nop.name:
                        del bb.instructions[i]
                        return nop
        raise RuntimeError("fresh nop not found")

    for fn in nc.m.functions:
        for bb in fn.blocks:
            insertions = []
            for idx, ins in enumerate(bb.instructions):
                if type(ins).__name__ in skip:
                    continue
                si = ins.sync_info
                if si is None or len(si.on_wait) <= 1:
                    continue
                waits = list(si.on_wait)
                nops = []
                for w in waits[:-1]:
                    nop = fresh_nop(str(ins.engine))
                    nop.sync_info = mybir.SyncInfo(on_wait=[w], on_update=[])
                    nops.append(nop)
                ins.sync_info = mybir.SyncInfo(
                    on_wait=[waits[-1]], on_update=list(si.on_update)
                )
                insertions.append((idx, nops))
            for idx, nops in reversed(insertions):
                bb.instructions[idx:idx] = nops


def _in_maps(inputs):
    x = np.ascontiguousarray(inputs["x"], np.float32)
    maps = []
    ident = np.eye(128, dtype=np.float32)
    gexp = np.zeros((HL, DL), np.float32)
    for h in range(HL):
        gexp[h, h * HD:(h + 1) * HD] = 1.0
    for c in range(NCORES):
        b, g = c // HG, c % HG
        sl = slice(g * DL, (g + 1) * DL)
        hsl = slice(g * HL, (g + 1) * HL)
        wq = inputs["Wq"][:, sl].reshape(DT, 128, DL).transpose(1, 0, 2).reshape(128, DT * DL)
        wk = inputs["Wk"][:, sl].reshape(DT, 128, DL).transpose(1, 0, 2).reshape(128, DT * DL)
        wv = inputs["Wv"][:, sl].reshape(DT, 128, DL).transpose(1, 0, 2).reshape(128, DT * DL)
        wo = inputs["Wo"][sl, :].reshape(2, 128, D).transpose(1, 0, 2).reshape(128, 2 * D)
        wes = np.concatenate([inputs["We"][:, hsl], inputs["Ws"][:, hsl]], axis=1)  # [1024, 8]
        wes = wes.reshape(DT, 128, 2 * HL).transpose(1, 0, 2).reshape(128, DT * 2 * HL)
        bes = np.stack([inputs["be"][hsl], inputs["bs"][hsl]], axis=1)  # [4, 2]
        maps.append({
            "x": np.ascontiguousarray(x[b]),
            "wq": np.ascontiguousarray(wq),
            "wk": np.ascontiguousarray(wk),
            "wv": np.ascontiguousarray(wv),
            "wo": np.ascontiguousarray(wo),
            "bq2": np.ascontiguousarray(inputs["bq"][sl].reshape(2, 128).T),
            "bk2": np.ascontiguousarray(inputs["bk"][sl].reshape(2, 128).T),
            "bvrow": np.ascontiguousarray(inputs["bv"][sl].reshape(1, DL)),
            "wes": np.ascontiguousarray(wes),
            "bes": np.ascontiguousarray(bes),
            "gexp": gexp,
            "ident": ident,
        })
    return maps


def kernel(**inputs):
    if "nc" not in _CACHE:
        _CACHE["nc"] = _build_nc()
    nc = _CACHE["nc"]
    maps = _in_maps({k: np.asarray(v) for k, v in inputs.items()})

    from concourse.bass_utils import run_bass_kernel_spmd

    trace = bool(int(os.environ.get("KERNEL_TRACE", "0")))
    res = run_bass_kernel_spmd(
        nc, maps, list(range(NCORES)), trace=trace,
        tmpdir=os.environ.get("KERNEL_TRACE_DIR") if trace else None,
    )
    _CACHE["last_result"] = res
    bo = np.asarray(inputs["bo"], np.float32)
    out = np.zeros((B, S, D), np.float32)
    for b in range(B):
        acc = np.zeros((S, D), np.float32)
        for g in range(HG):
            acc += res.results[b * HG + g]["out"]
        out[b] = acc + bo
    return out
